# revision 13
# baseline (speedup 1.0000x reference)
"""Trainium2 Bass kernel for nn_Attention_84224308674843.

Single-head attention with tanh-squashed scores and a length-masked softmax
renormalization:

    kx   = k @ W_k                      [B, KL, H]
    qw   = q @ (W_q @ weight)           [B, QL, H]
    S    = tanh(qw @ kx^T)              [B, QL, KL]
    attn = masked_softmax(S, len)       (softmax -> mask -> renorm == masked softmax)
    out  = (attn @ kx) @ proj_w.T + proj_b

Host folds the weight chain on both sides of the attention matrix:
    W_s = (W_q @ weight) @ W_k^T   ->  S = (q @ W_s) @ k^T
    W_o = W_k @ proj_w^T           ->  out = (attn @ k) @ W_o
so the device never materializes kx and does 21.5 GF/batch instead of 30.

Device per batch: PE-transpose k/q to kT/qT, qsT = W_s^T.qT, S chunks with
tanh/exp on ACT + masked renorm on DVE, attn transposed on PE, GT = attn@k
(k streamed from a bf16 DRAM scratch), final = GT^T @ W_o. All matmuls bf16
with fp32 PSUM accumulation.

Sharding: data-parallel over batch, 2 batches per core on 8 cores.
"""

import os
import sys
from contextlib import ExitStack

import numpy as np

for _p in ("/opt/trn_rl_repo", os.path.expanduser("~/.axon_site/_ro/trn_rl_repo")):
    if os.path.isdir(_p) and _p not in sys.path:
        sys.path.insert(0, _p)

import ml_dtypes

import concourse.bass as bass
import concourse.mybir as mybir
import concourse.tile as tile
from concourse import bacc
from concourse.bass_utils import run_bass_kernel_spmd
from concourse.masks import make_identity

BF16 = mybir.dt.bfloat16
F32 = mybir.dt.float32
AF = mybir.ActivationFunctionType
ALU = mybir.AluOpType

B, KL, QL, E, H = 16, 4096, 1024, 1024, 1024
N_CORES = 8
PB = B // N_CORES  # batches per core
P = 128
FD = 512  # matmul moving free dim / chunk width
TG = 4   # transpose batch group (s-tiles per eviction)


def build_program(pb=PB, kl=KL, ql=QL, e=E, timing_variant=False):
    """Build the per-core Bass program (all cores run it on different data).

    timing_variant=True: big I/O tensors become internal DRAM (same compute
    and DMA traffic); only a 1-element dummy in/out pair crosses the host
    boundary, so repeated executions measure device time, not transfers.
    """
    n_st = kl // P   # s (key position) tiles
    n_qt = ql // P   # q tiles
    n_et = e // P    # e (feature) tiles
    n_ck = kl // FD  # 512-wide chunks along s
    n_qh = ql // FD  # 512-wide chunks along q
    n_eh = e // FD   # 512-wide chunks along e
    tp_per_fd = FD // P

    nc = bacc.Bacc("TRN2", target_bir_lowering=False, debug=False)

    if timing_variant:
        k2 = nc.dram_tensor("k2", [pb, kl, e], F32)
        q2 = nc.dram_tensor("q2", [pb, ql, e], F32)
        mask2 = nc.dram_tensor("mask2", [pb, kl], BF16)
        w_s = nc.dram_tensor("w_s", [e, e], BF16)
        w_o = nc.dram_tensor("w_o", [e, e], BF16)
        out2 = nc.dram_tensor("out2", [pb, ql, e], F32)
        attn2 = nc.dram_tensor("attn2", [pb, ql, kl], F32)
        din = nc.declare_dram_parameter("din", [1, 1], F32, isOutput=False)
        dout = nc.declare_dram_parameter("dout", [1, 1], F32, isOutput=True)
    else:
        k2 = nc.declare_dram_parameter("k2", [pb, kl, e], F32, isOutput=False)
        q2 = nc.declare_dram_parameter("q2", [pb, ql, e], F32, isOutput=False)
        mask2 = nc.declare_dram_parameter("mask2", [pb, kl], BF16, isOutput=False)
        w_s = nc.declare_dram_parameter("w_s", [e, e], BF16, isOutput=False)
        w_o = nc.declare_dram_parameter("w_o", [e, e], BF16, isOutput=False)
        out2 = nc.declare_dram_parameter("out2", [pb, ql, e], F32, isOutput=True)
        attn2 = nc.declare_dram_parameter("attn2", [pb, ql, kl], F32, isOutput=True)

    def phase1_transpose_inputs(nc, po, b, ident):
        """Load k/q (cast f32->bf16 in DMA), write k bf16 scratch, transpose
        128x128 blocks on PE with group-of-TG batched ACT evictions."""
        ks = po["ks"].tile([kl, e], BF16, tag="ks", name=f"ks{b}")
        kt = [
            po["kT"].tile([P, kl], BF16, tag="kT", name=f"kT{b}_{j}")
            for j in range(n_et)
        ]
        for ig in range(n_st // TG):
            klds = []
            for t in range(TG):
                i = ig * TG + t
                kld = po["ld"].tile([P, e], BF16, tag="ld", name=f"kld{b}_{i}")
                nc.gpsimd.dma_start(kld[:], k2[b, i * P : (i + 1) * P, :])
                nc.sync.dma_start(ks[i * P : (i + 1) * P, :], kld[:])
                klds.append(kld)
            for j in range(n_et):
                tp = po["tps"].tile([P, TG * P], BF16, tag="tps", name=f"ktp{b}_{ig}_{j}")
                for t in range(TG):
                    nc.tensor.transpose(
                        tp[:, t * P : (t + 1) * P],
                        klds[t][:, j * P : (j + 1) * P],
                        ident[:],
                    )
                nc.scalar.activation(
                    kt[j][:, ig * TG * P : (ig + 1) * TG * P], tp[:], AF.Copy
                )
        qt = [
            po["qT"].tile([P, ql], BF16, tag="qT", name=f"qT{b}_{j}")
            for j in range(n_et)
        ]
        for ig in range(n_qt // TG):
            qlds = []
            for t in range(TG):
                i = ig * TG + t
                qld = po["ld"].tile([P, e], BF16, tag="ld", name=f"qld{b}_{i}")
                nc.gpsimd.dma_start(qld[:], q2[b, i * P : (i + 1) * P, :])
                qlds.append(qld)
            for j in range(n_et):
                tp = po["tps"].tile([P, TG * P], BF16, tag="tps", name=f"qtp{b}_{ig}_{j}")
                for t in range(TG):
                    nc.tensor.transpose(
                        tp[:, t * P : (t + 1) * P],
                        qlds[t][:, j * P : (j + 1) * P],
                        ident[:],
                    )
                nc.scalar.activation(
                    qt[j][:, ig * TG * P : (ig + 1) * TG * P], tp[:], AF.Copy
                )
        return ks, kt, qt

    def phase2_qst(nc, po, b, qt):
        """qsT[ek, q] = sum_e W_s[e, ek] qT[e, q] (contract e)."""
        wst = [
            po["wt"].tile([P, e], BF16, tag="wt", name=f"ws{b}_{i}")
            for i in range(n_et)
        ]
        for i in range(n_et):
            nc.sync.dma_start(wst[i][:], w_s[i * P : (i + 1) * P, :])
        qst = [
            po["qsT"].tile([P, ql], BF16, tag="qsT", name=f"qsT{b}_{j}")
            for j in range(n_et)
        ]
        for ek in range(n_et):
            for qh in range(n_qh):
                ps = po["mm"].tile([P, FD], F32, tag="mm", name=f"psq{b}_{ek}_{qh}")
                for ee in range(n_et):
                    nc.tensor.matmul(
                        ps[:],
                        wst[ee][:, ek * P : (ek + 1) * P],
                        qt[ee][:, qh * FD : (qh + 1) * FD],
                        start=(ee == 0),
                        stop=(ee == n_et - 1),
                    )
                nc.scalar.activation(
                    qst[ek][:, qh * FD : (qh + 1) * FD], ps[:], AF.Copy
                )
        return qst

    def make_mask(nc, po, b):
        mask_t = po["mask"].tile([P, kl], BF16, tag="mask", name=f"mask{b}")
        nc.sync.dma_start(mask_t[:1, :], mask2[b : b + 1, :])
        nc.gpsimd.partition_broadcast(mask_t[:], mask_t[:1, :])
        return mask_t

    def phase4_scores_softmax(nc, po, b, qh, kt, qst, mask_t, ident):
        """S chunks -> tanh -> exp -> mask*renorm -> attn out + attnT tiles.

        Covers q tiles [qh*tp_per_fd, (qh+1)*tp_per_fd); attnT is FD q wide.
        """
        attnt = [
            po["attnT"].tile([P, FD], BF16, tag="attnT", name=f"attnT{b}_{qh}_{s}")
            for s in range(n_st)
        ]
        for qtj in range(tp_per_fd):
            qti = qh * tp_per_fd + qtj
            mslab = po["soft"].tile([P, kl], BF16, tag="soft", name=f"M{b}_{qti}")
            zcol = po["zr"].tile([P, n_ck], F32, tag="zc", bufs=2, name=f"Zc{b}_{qti}")
            for c in range(n_ck):
                ps = po["mm"].tile([P, FD], F32, tag="mm", name=f"pss{b}_{qti}_{c}")
                for ek in range(n_et):
                    nc.tensor.matmul(
                        ps[:],
                        qst[ek][:, qti * P : (qti + 1) * P],
                        kt[ek][:, c * FD : (c + 1) * FD],
                        start=(ek == 0),
                        stop=(ek == n_et - 1),
                    )
                tcnk = po["ev"].tile([P, FD], BF16, tag="tc", bufs=2, name=f"T{b}_{qti}_{c}")
                nc.scalar.activation(tcnk[:], ps[:], AF.Tanh)
                ucnk = po["ev"].tile([P, FD], BF16, tag="uc", bufs=2, name=f"U{b}_{qti}_{c}")
                nc.scalar.activation(ucnk[:], tcnk[:], AF.Exp)
                nc.vector.tensor_mul(
                    mslab[:, c * FD : (c + 1) * FD],
                    ucnk[:],
                    mask_t[:, c * FD : (c + 1) * FD],
                )
                nc.vector.reduce_sum(
                    zcol[:, c : c + 1],
                    mslab[:, c * FD : (c + 1) * FD],
                    axis=mybir.AxisListType.X,
                )
            z = po["zr"].tile([P, 1], F32, tag="z", name=f"Z{b}_{qti}")
            nc.vector.reduce_sum(z[:], zcol[:], axis=mybir.AxisListType.X)
            r = po["zr"].tile([P, 1], F32, tag="r", bufs=2, name=f"R{b}_{qti}")
            nc.vector.reciprocal(r[:], z[:])
            for c in range(n_ck):
                a32 = po["ev"].tile([P, FD], F32, tag="a32", bufs=2, name=f"A32{b}_{qti}_{c}")
                nc.vector.tensor_scalar_mul(
                    a32[:], mslab[:, c * FD : (c + 1) * FD], r[:]
                )
                nc.sync.dma_start(
                    attn2[b, qti * P : (qti + 1) * P, c * FD : (c + 1) * FD],
                    a32[:],
                )
                a16 = po["ev"].tile([P, FD], BF16, tag="a16", bufs=2, name=f"A16{b}_{qti}_{c}")
                nc.vector.tensor_scalar_mul(
                    a16[:], mslab[:, c * FD : (c + 1) * FD], r[:]
                )
                for j in range(tp_per_fd):
                    s_tile = c * tp_per_fd + j
                    tp = po["tps"].tile(
                        [P, P], BF16, tag="tps", name=f"atp{b}_{qti}_{s_tile}"
                    )
                    nc.tensor.transpose(tp[:], a16[:, j * P : (j + 1) * P], ident[:])
                    dst = attnt[s_tile][:, qtj * P : (qtj + 1) * P]
                    if s_tile % 2 == 0:
                        nc.vector.tensor_copy(dst, tp[:])
                    else:
                        nc.scalar.activation(dst, tp[:], AF.Copy)
        return attnt

    def phase5_gt(nc, po, b, qh, ks, attnt, gt):
        """GT[e, q-half] = sum_s k[s, e] attn[q, s] (contract s, stream k)."""
        for eg in range(n_eh):
            pss = [
                po["mm"].tile([P, FD], F32, tag="mm", name=f"psg{b}_{eg}_{qh}_{ej}")
                for ej in range(tp_per_fd)
            ]
            for i in range(n_st):
                kblk = po["kld5"].tile(
                    [P, FD], BF16, tag="kld5", name=f"kb{b}_{eg}_{qh}_{i}"
                )
                nc.sync.dma_start(
                    kblk[:], ks[i * P : (i + 1) * P, eg * FD : (eg + 1) * FD]
                )
                for ej in range(tp_per_fd):
                    nc.tensor.matmul(
                        pss[ej][:],
                        kblk[:, ej * P : (ej + 1) * P],
                        attnt[i][:],
                        start=(i == 0),
                        stop=(i == n_st - 1),
                    )
            for ej in range(tp_per_fd):
                nc.scalar.activation(
                    gt[eg * tp_per_fd + ej][:, qh * FD : (qh + 1) * FD],
                    pss[ej][:],
                    AF.Copy,
                )

    def phase6_final(nc, po, b, gt):
        """final[q, e'] = sum_e GT[e, q] W_o[e, e'] -> out2."""
        wot = [
            po["wt"].tile([P, e], BF16, tag="wt", name=f"wo{b}_{i}")
            for i in range(n_et)
        ]
        for i in range(n_et):
            nc.sync.dma_start(wot[i][:], w_o[i * P : (i + 1) * P, :])
        for qti in range(n_qt):
            for eh in range(n_eh):
                ps = po["mm"].tile([P, FD], F32, tag="mm", name=f"psf{b}_{qti}_{eh}")
                for ej in range(n_et):
                    nc.tensor.matmul(
                        ps[:],
                        gt[ej][:, qti * P : (qti + 1) * P],
                        wot[ej][:, eh * FD : (eh + 1) * FD],
                        start=(ej == 0),
                        stop=(ej == n_et - 1),
                    )
                fin = po["ev"].tile([P, FD], F32, tag="fin", bufs=2, name=f"fin{b}_{qti}_{eh}")
                nc.scalar.activation(fin[:], ps[:], AF.Copy)
                nc.sync.dma_start(
                    out2[b, qti * P : (qti + 1) * P, eh * FD : (eh + 1) * FD],
                    fin[:],
                )

    with ExitStack() as ctx:
        tc = ctx.enter_context(tile.TileContext(nc))
        pool_specs = [
            ("const", 1, "SBUF"),
            ("ld", 5, "SBUF"),
            ("kT", n_et, "SBUF"),
            ("qT", n_et, "SBUF"),
            ("qsT", n_et, "SBUF"),
            ("wt", 8, "SBUF"),
            ("mask", 1, "SBUF"),
            ("soft", 1, "SBUF"),
            ("zr", 2 * (n_ck + 1), "SBUF"),
            ("attnT", n_st, "SBUF"),
            ("outT", n_et, "SBUF"),
            ("kld5", 3, "SBUF"),
            ("ev", 2, "SBUF"),
            ("ks", 2, "DRAM"),
            ("tps", 2, "PSUM"),
            ("mm", 6, "PSUM"),
        ]
        po = {
            name: ctx.enter_context(tc.tile_pool(name=name, bufs=bufs, space=space))
            for name, bufs, space in pool_specs
        }

        ident = po["const"].tile([P, P], BF16, tag="ident", name="ident")
        make_identity(nc, ident[:])

        for b in range(pb):
            ks, kt, qt = phase1_transpose_inputs(nc, po, b, ident)
            qst = phase2_qst(nc, po, b, qt)
            mask_t = make_mask(nc, po, b)
            gt = [
                po["outT"].tile([P, ql], BF16, tag="outT", name=f"gt{b}_{j}")
                for j in range(n_et)
            ]
            for qh in range(n_qh):
                attnt = phase4_scores_softmax(nc, po, b, qh, kt, qst, mask_t, ident)
                phase5_gt(nc, po, b, qh, ks, attnt, gt)
            phase6_final(nc, po, b, gt)

        if timing_variant:
            nc.sync.dma_start(dout[:], din[:])

    nc.compile()
    return nc


_NC_CACHE = {}


def _get_nc():
    if "nc" not in _NC_CACHE:
        _NC_CACHE["nc"] = build_program()
    return _NC_CACHE["nc"]


def make_device_inputs(k, q, memory_len, w_kx, w_qx, weight, proj_w):
    """Host-side weight folding + per-core input maps."""
    bf = ml_dtypes.bfloat16
    k = np.ascontiguousarray(np.asarray(k, dtype=np.float32))
    q = np.ascontiguousarray(np.asarray(q, dtype=np.float32))

    w_k_f = np.asarray(w_kx[0], dtype=np.float32)
    w_qw = np.asarray(w_qx[0], dtype=np.float32) @ np.asarray(weight, dtype=np.float32)
    w_s16 = (w_qw @ w_k_f.T).astype(bf)
    w_o16 = (w_k_f @ np.asarray(proj_w, dtype=np.float32).T).astype(bf)
    mask16 = (
        (np.arange(KL)[None, :] < np.asarray(memory_len)[:, None])
        .astype(np.float32)
        .astype(bf)
    )
    in_maps = []
    for c in range(N_CORES):
        lo, hi = c * PB, (c + 1) * PB
        in_maps.append(
            {
                "k2": k[lo:hi],
                "q2": q[lo:hi],
                "mask2": np.ascontiguousarray(mask16[lo:hi]),
                "w_s": w_s16,
                "w_o": w_o16,
            }
        )
    return in_maps


def kernel(k, q, memory_len, w_kx, w_qx, weight, proj_w, proj_b):
    nc = _get_nc()
    in_maps = make_device_inputs(k, q, memory_len, w_kx, w_qx, weight, proj_w)
    res = run_bass_kernel_spmd(nc, in_maps, list(range(N_CORES)))
    out = np.concatenate([res.results[c]["out2"] for c in range(N_CORES)], axis=0)
    attn = np.concatenate([res.results[c]["attn2"] for c in range(N_CORES)], axis=0)
    out = out + np.asarray(proj_b, dtype=np.float32)[None, None, :]
    return out.astype(np.float32), attn.astype(np.float32)
